# revision 58
# baseline (speedup 1.0000x reference)
"""C3DLoss kernel for Trainium2 — 8-core batch-parallel, raw-Bass implementation.

Per core = one batch frame b (pairing partner tb = b^1):
    partial = sum over terms t in {same, cross}, shifts d in [-2,2]^2, pixels p:
        exp(-50 * |feat_r(p) - feat_q(p+d)|^2)
    feat = (x, y, z', r, g, b) with the masks FOLDED INTO z:
        z'_ref   = z_ref   - 120*(1 - mask_ref)
        z'_query = z_query -  40*(1 - mask_query)
    With z in (0, 10.2], any off-mask combination leaves |dz| >= ~30, so
    d2 >= ~900 and exp(-50*d2) = 0 exactly like the reference's mask
    product; with both masks on, z' = z exactly.
    loss = -(sum of partials - garbage_const) / max(sum(depth_gt_mask), 1).

Device mapping (v5):
  - Host pre-blocks every plane into G=21 W-blocks of width 58 (the last
    block right-padded; dead ref columns are killed by padding ref z' with
    -120) with +-2 halo in both dims, fp16. 6 channels x 21 blocks = 126
    partitions in ONE tile — G maxed against the 128-partition limit, which
    minimizes the per-op free size (32 rows x 62 = 1984).
  - The 5 dx-shifts of each (term, dy) group are computed by ONE batched DVE
    subtract (fp16 2x mode, ref broadcast via stride-0 free dim) into a
    5-slot mega tile; squares run per contiguous slot-run, statically split
    across DVE / ACT(Square) / Pool(GPSIMD) using HW-measured engine rates
    (see calibrate.py: DVE 1110ns, ACT 2224ns, Pool 3753ns per 2176-elem
    slot).
  - PE reduces channels with [126, 32] selector weights (cols 21..31 zero);
    4 slots pack a 128-partition PSUM bank set; chunks are 8 rows x 464
    cols per PSUM bank, 4 banks per 4-slot batch, so batches alternate
    cleanly between bank sets {0-3} and {4-7}.
  - ACT computes exp(-50*d2) in place on PSUM, one op per batch over all 4
    banks ([pb, 4, 464] bank-strided), accum into acc[:, batch]. Zero-weight
    output partitions contribute exp(0)=1 each; that deterministic constant
    is subtracted on the host.
"""

import sys

for _p in ("/opt/trn_rl_repo", "/opt/pypackages"):
    if _p not in sys.path:
        sys.path.insert(0, _p)

from contextlib import ExitStack

import numpy as np

import concourse.bass as bass
import concourse.mybir as mybir
from concourse.ap import AP
from concourse.alu_op_type import AluOpType

F16 = mybir.dt.float16
F32 = mybir.dt.float32

R = 2
G = 21            # W-blocks (last block right-padded: 21*58 = 1218 >= 1216)
CH = 6            # x, y, z', r, g, b
P = CH * G        # 126 partitions per feature tile
SBATCH = 4        # slots per PSUM bank set
GRP = 5           # dx-shifts batched per DVE subtract (one (t, dy) group)
NSQG = 6          # rotating 5-slot diff/square mega tiles
MREF_C = 120.0    # ref-side mask fold magnitude
MQ_C = 40.0       # query-side mask fold magnitude
EXP_SCALE = -50.0

# per-slab square-engine assignment counts (DVE, ACT, Pool), balanced
# against HW-measured rates (calibrate.py): DVE 1110ns, ACT 2224ns,
# Pool 3753ns per [114, 2176] slot
N_SQ_DVE = 14
N_SQ_ACT = 18
N_SQ_POOL = 18


class Cfg:
    def __init__(self, H=352, W=1216, HS=32):
        assert H % HS == 0
        self.H, self.W, self.HS = H, W, HS
        self.WB = (W + G - 1) // G             # 58 (last block padded)
        self.WP = G * self.WB                  # 1218 padded width
        self.WBH = self.WB + 2 * R             # 62
        self.Hp = H + 2 * R                    # 356
        self.NSLAB = H // HS                   # 11
        self.NQ = G * self.Hp * self.WBH       # haloed plane elems
        self.QF = (HS + 2 * R) * self.WBH      # query tile free size 2232
        self.SF = HS * self.WBH                # slab tile free size 1984
        cr = 512 // self.WB                    # 8 rows -> 464 cols
        assert HS % cr == 0
        self.CC = cr * self.WB                 # 464 psum cols per chunk
        self.chunks = [(o, cr) for o in range(0, HS, cr)]
        self.NC = len(self.chunks)             # 4
        assert self.NC * 2 == 8                # two alternating bank sets
        self.slots = [(t, dy, dx) for t in (0, 1)
                      for dy in range(-R, R + 1) for dx in range(-R, R + 1)]
        self.NS = len(self.slots)              # 50
        self.batches = [self.slots[i:i + SBATCH]
                        for i in range(0, self.NS, SBATCH)]
        self.NB = len(self.batches)            # 13
        self.GB = self.NSLAB * self.NB         # 143 global batches
        self.NEXP = self.GB                    # one exp op per batch
        self.NSLOT = self.NSLAB * self.NS

        # square-engine assignment per slab slot (largest-remainder
        # interleave), then sorted within each 5-slot (t, dy) group so each
        # engine's slots form one contiguous run per group.
        want = {'D': N_SQ_DVE, 'A': N_SQ_ACT, 'P': N_SQ_POOL}
        assert sum(want.values()) == self.NS
        emitted = {k: 0 for k in want}
        raw = []
        for j in range(self.NS):
            k = max(want, key=lambda e: want[e] * (j + 1) / self.NS - emitted[e])
            emitted[k] += 1
            raw.append(k)
        order = {'D': 0, 'A': 1, 'P': 2}
        self.assign = []
        self.NGS = self.NS // GRP              # groups per slab (10)
        self.runs = []                         # per slab-group: [(eng, k0, k1)]
        for gsl in range(self.NGS):
            grp = sorted(raw[gsl * GRP:(gsl + 1) * GRP], key=order.get)
            self.assign.extend(grp)
            rr = []
            for k, e in enumerate(grp):
                if rr and rr[-1][0] == e:
                    rr[-1][2] = k
                else:
                    rr.append([e, k, k])
            self.runs.append([tuple(x) for x in rr])
        self.NGRP = self.NSLAB * self.NGS      # 110 global groups
        # per-slot square-done thresholds: count of engine e's increments
        # visible once the run containing this slot completes
        self.thr = []
        tot = {'D': 0, 'A': 0, 'P': 0}
        for gj in range(self.NSLOT):
            j = gj % self.NS
            gsl, k = divmod(j, GRP)
            run = next(r for r in self.runs[gsl] if r[1] <= k <= r[2])
            e = run[0]
            if k == run[1]:                    # run start: bump totals now
                tot[e] += run[2] - run[1] + 1
            self.thr.append((e, tot[e]))

    def garbage_const(self):
        """exp(0)=1 contributions from the (32-G) zero-weight output
        partitions of each 32-partition slot group; each exp op reads
        NC*CC free elements per partition."""
        tot = 0
        for gb in range(self.GB):
            nslots = len(self.batches[gb % self.NB])
            tot += nslots * (32 - G) * self.NC * self.CC
        return float(tot)


def make_selw():
    s = np.zeros((P, 32), dtype=np.float16)
    for c in range(CH):
        for g in range(G):
            s[c * G + g, g] = 1.0
    return s


def _apv(t_ap, pcnt, free_dims, free_off=0):
    pstride = t_ap.ap[0][0]
    return AP(t_ap.tensor, t_ap.offset + free_off,
              [[pstride, pcnt]] + [list(d) for d in free_dims])


def _dram_ap(handle, offset, dims):
    a = handle[:]
    return AP(a.tensor, a.offset + offset, [list(d) for d in dims])


def emit(nc: bass.Bass, cfg: Cfg):
    HS, WB, WBH, Hp = cfg.HS, cfg.WB, cfg.WBH, cfg.Hp
    NQ, QF, SF = cfg.NQ, cfg.QF, cfg.SF
    NSLAB, NB, NC, NS = cfg.NSLAB, cfg.NB, cfg.NC, cfg.NS
    Act = mybir.ActivationFunctionType

    dp = nc.declare_dram_parameter
    q_d = dp("q_d", [2, CH, NQ], F16, isOutput=False)    # query planes per term
    r_d = dp("r_d", [2, CH, NQ], F16, isOutput=False)    # ref planes per term
    selw_d = dp("selw_d", [P, 32], F16, isOutput=False)
    out_d = dp("out_d", [128, 1], F32, isOutput=True)

    with ExitStack() as ex:
        E = ex.enter_context
        q_s = [[E(nc.sbuf_tensor(f"q{t}{p}", [P, QF + 2 * R], F16))
                for p in range(2)] for t in range(2)]
        r_s = [[E(nc.sbuf_tensor(f"r{t}{p}", [P, SF], F16))
                for p in range(2)] for t in range(2)]
        sq_s = [E(nc.sbuf_tensor(f"sq{i}", [P, GRP * SF], F16))
                for i in range(NSQG)]
        acc_s = E(nc.sbuf_tensor("acc", [128, cfg.NEXP], F32))
        res_s = E(nc.sbuf_tensor("res", [128, 1], F32))
        selw_s = E(nc.sbuf_tensor("selw", [P, 32], F16))
        ps_s = E(nc.psum_tensor("ps", [128, 4096], F32))

        sLC = E(nc.semaphore("sLC"))
        sLF = E(nc.semaphore("sLF"))  # slab 0 t=0 tiles (early start)
        sL0 = E(nc.semaphore("sL0"))
        sL1 = E(nc.semaphore("sL1"))
        sG = E(nc.semaphore("sG"))
        sV = E(nc.semaphore("sV"))    # DVE subs (+ final reduce)
        sQD = E(nc.semaphore("sQD"))  # DVE squares
        sQA = E(nc.semaphore("sQA"))  # ACT squares
        sQP = E(nc.semaphore("sQP"))  # Pool squares
        sP = E(nc.semaphore("sP"))    # PE matmul completions (4 per slot)
        sE = E(nc.semaphore("sE"))    # exp ops done (1 per batch)
        blk = E(nc.Block())

        sqsem = {'D': sQD, 'A': sQA, 'P': sQP}

        def group_sub_aps(s, gsl):
            # group gsl covers slots (t, dy, dx=-2..2); one batched subtract
            # over useful columns only (halo cols skipped via row stride)
            t, dy, _ = cfg.slots[gsl * GRP]
            ph = s % 2
            g = s * cfg.NGS + gsl
            tile = sq_s[g % NSQG].ap()
            sq = AP(tile.tensor, tile.offset + 2,
                    [[tile.ap[0][0], P], [SF, GRP], [WBH, HS], [1, WB]])
            rt = r_s[t][ph].ap()
            r = AP(rt.tensor, rt.offset + 2,
                   [[rt.ap[0][0], P], [0, GRP], [WBH, HS], [1, WB]])
            qt = q_s[t][ph].ap()
            q = AP(qt.tensor, qt.offset + (2 + dy) * WBH,
                   [[qt.ap[0][0], P], [1, GRP], [WBH, HS], [1, WB]])
            return sq, r, q

        def sq_slice(gj, length=1):
            g, k = divmod(gj, GRP)
            tile = sq_s[g % NSQG].ap()
            return AP(tile.tensor, tile.offset + k * SF + 2,
                      [[tile.ap[0][0], P], [SF, length], [WBH, HS], [1, WB]])

        # ---------------- SP: DMA ----------------
        @blk.sync
        def _(sp):
            sp.dma_start(selw_s[:], selw_d[:]).then_inc(sLC, 16)
            for s in range(NSLAB):
                ph = s % 2
                r0 = s * HS
                sLs = sL0 if s % 2 == 0 else sL1
                if s >= 2:
                    sp.wait_ge(sV, cfg.NGS * (s - 1))
                for t in range(2):
                    sem = sLF if (s == 0 and t == 0) else sLs
                    sp.dma_start(
                        _apv(q_s[t][ph].ap(), P, [[1, QF]]),
                        _dram_ap(q_d, t * CH * NQ + r0 * WBH,
                                 [[NQ, CH], [Hp * WBH, G], [1, QF]])
                    ).then_inc(sem, 16)
                    sp.dma_start(
                        r_s[t][ph].ap(),
                        _dram_ap(r_d, t * CH * NQ + (r0 + 2) * WBH,
                                 [[NQ, CH], [Hp * WBH, G], [1, SF]])
                    ).then_inc(sem, 16)
            sp.wait_ge(sV, cfg.NGRP + 1)
            sp.dma_start(out_d[:], res_s.ap()).then_inc(sLC, 16)

        # ---------------- DVE: batched subs + its square runs + reduce ----
        @blk.vector
        def _(ve):
            ve.wait_ge(sG, 1)
            for s in range(NSLAB):
                if s == 0:
                    ve.wait_ge(sLF, 32)              # t=0 tiles only
                elif s % 2 == 0:
                    ve.wait_ge(sL0, 32 + 64 * (s // 2))
                else:
                    ve.wait_ge(sL1, 64 * (s // 2 + 1))
                for gsl in range(cfg.NGS):
                    if s == 0 and gsl == cfg.NGS // 2:
                        ve.wait_ge(sL0, 32)          # t=1 tiles of slab 0
                    g = s * cfg.NGS + gsl
                    if g >= NSQG:
                        # previous user of this mega tile fully consumed
                        ve.wait_ge(sP, NC * GRP * (g - NSQG + 1))
                    sq, r, q = group_sub_aps(s, gsl)
                    nc.vector.tensor_tensor(
                        sq, r, q, AluOpType.subtract).then_inc(sV, 1)
                    for (e, k0, k1) in cfg.runs[gsl]:
                        if e != 'D':
                            continue
                        ve.wait_ge(sV, g + 1)
                        a = sq_slice(g * GRP + k0, k1 - k0 + 1)
                        nc.vector.tensor_mul(a, a, a).then_inc(
                            sQD, k1 - k0 + 1)
            ve.wait_ge(sE, cfg.NEXP)
            nc.vector.tensor_reduce(
                res_s.ap(), acc_s.ap(), axis=mybir.AxisListType.X,
                op=AluOpType.add).then_inc(sV, 1)

        # ---------------- PE: selector matmuls ----------------
        @blk.tensor
        def _(pe):
            pe.wait_ge(sLC, 16)
            for gb in range(cfg.GB):
                s, b = divmod(gb, NB)
                bslots = cfg.batches[b]
                if gb >= 2:
                    pe.wait_ge(sE, gb - 1)   # bank set free (exp of gb-2)
                k0 = (gb * NC) % 8
                for jj in range(len(bslots)):
                    j = b * SBATCH + jj
                    gj = s * NS + j
                    e, thr = cfg.thr[gj]
                    pe.wait_ge(sqsem[e], thr)
                    tile = sq_slice(gj)
                    pstride = tile.ap[0][0]
                    for c, (ro, nr) in enumerate(cfg.chunks):
                        k = k0 + c
                        # tile.offset already includes the +2 halo skip
                        rhs = AP(tile.tensor, tile.offset + ro * WBH,
                                 [[pstride, P], [WBH, nr], [1, WB]])
                        nc.tensor.matmul(
                            ps_s[32 * jj:32 * jj + 32,
                                 512 * k:512 * k + cfg.CC],
                            selw_s[:], rhs, start=True, stop=True,
                            skip_group_check=True, tile_position=(0, 32 * jj)
                        ).then_inc(sP, 1)

        # ---------------- ACT: its squares (2-batch lookahead) + exps ------
        @blk.scalar
        def _(ac):
            ac.wait_ge(sG, 1)
            ps_ap = ps_s.ap()
            ps_pstride = ps_ap.ap[0][0]
            LOOK = 2   # batches of square lookahead ahead of the exp stream
            for gbi in range(-LOOK, cfg.GB):
                gb2 = gbi + LOOK
                if gb2 < cfg.GB:
                    s, b = divmod(gb2, NB)
                    # emit A-runs of groups whose first slot lands in batch gb2
                    for gsl in range(cfg.NGS):
                        if (gsl * GRP) // SBATCH != b:
                            continue
                        g = s * cfg.NGS + gsl
                        for (e, k0, k1) in cfg.runs[gsl]:
                            if e != 'A':
                                continue
                            ac.wait_ge(sV, g + 1)
                            a = sq_slice(g * GRP + k0, k1 - k0 + 1)
                            nc.scalar.activation(
                                a, a, Act.Square).then_inc(sQA, k1 - k0 + 1)
                if gbi >= 0:
                    gb = gbi
                    s_, b_ = divmod(gb, NB)
                    nslots = len(cfg.batches[b_])
                    last_gj = s_ * NS + b_ * SBATCH + nslots - 1
                    ac.wait_ge(sP, NC * (last_gj + 1))
                    k0 = (gb * NC) % 8
                    pb = 32 * nslots
                    pa = AP(ps_ap.tensor, ps_ap.offset + 512 * k0,
                            [[ps_pstride, pb], [512, NC], [1, cfg.CC]])
                    nc.scalar.activation(
                        pa, pa, Act.Exp, scale=EXP_SCALE,
                        accum_out=acc_s[:pb, gb:gb + 1]).then_inc(sE, 1)

        # ---------------- Pool: init memsets + its squares ----------------
        @blk.gpsimd
        def _(gp):
            gp.memset(acc_s.ap(), 0.0)
            for t in range(2):
                for p in range(2):
                    gp.memset(q_s[t][p][:, QF:QF + 2 * R], 0.0)
            gp.drain()
            gp.sem_inc(sG, 1)
            for g in range(cfg.NGRP):
                gsl = g % cfg.NGS
                for (e, k0, k1) in cfg.runs[gsl]:
                    if e != 'P':
                        continue
                    gp.wait_ge(sV, g + 1)
                    a = sq_slice(g * GRP + k0, k1 - k0 + 1)
                    nc.gpsimd.tensor_tensor(
                        a, a, a, AluOpType.mult).then_inc(sQP, k1 - k0 + 1)
    return nc


# ---------------- host side ----------------

def _block(plane, cfg, pad=0.0):
    """[H, W] -> flat blocked+haloed [G*Hp*WBH] fp16 (W padded to G*WB)."""
    p = np.full((cfg.Hp, cfg.WP + 2 * R), pad, dtype=np.float32)
    p[R:R + cfg.H, R:R + cfg.W] = plane
    out = np.empty((G, cfg.Hp, cfg.WBH), dtype=np.float16)
    for g in range(G):
        out[g] = p[:, g * cfg.WB:g * cfg.WB + cfg.WBH]
    return out.reshape(-1)


def host_precompute(rgb, depth, depth_gt, depth_mask, depth_gt_mask,
                    xy1_grid, Ts, cfg, b):
    tb = b ^ 1
    xy1 = np.asarray(xy1_grid[b], np.float32)
    xy1_t = np.asarray(xy1_grid[tb], np.float32)
    dep = np.asarray(depth[b, 0], np.float32)
    dgt_b = np.asarray(depth_gt[b, 0], np.float32)
    dgt_t = np.asarray(depth_gt[tb, 0], np.float32)
    mp = np.asarray(depth_mask[b, 0], np.float32)
    mg_b = np.asarray(depth_gt_mask[b, 0], np.float32)
    mg_t = np.asarray(depth_gt_mask[tb, 0], np.float32)

    xyz_p = xy1 * dep
    T21 = (np.linalg.inv(np.asarray(Ts[tb], np.float64)) @
           np.asarray(Ts[b], np.float64)).astype(np.float32)
    Rm, tv = T21[:3, :3], T21[:3, 3]
    txyz = np.einsum('ij,jhw->ihw', Rm, xyz_p).astype(np.float32) \
        + tv[:, None, None].astype(np.float32)
    pos = (txyz[2] > 0).astype(np.float32) * mp

    q = np.empty((2, CH, cfg.NQ), np.float16)
    r = np.empty((2, CH, cfg.NQ), np.float16)
    for c in range(2):
        q[0, c] = _block(xyz_p[c], cfg)
        q[1, c] = _block(txyz[c], cfg)
        r[0, c] = _block(xy1[c] * dgt_b, cfg)
        r[1, c] = _block(xy1_t[c] * dgt_t, cfg)
    # z' channels: mask folded in; query padding = mask-off (-MQ_C)
    q[0, 2] = _block(xyz_p[2] - MQ_C * (1.0 - mp), cfg, pad=-MQ_C)
    q[1, 2] = _block(txyz[2] - MQ_C * (1.0 - pos), cfg, pad=-MQ_C)
    r[0, 2] = _block(xy1[2] * dgt_b - MREF_C * (1.0 - mg_b), cfg,
                     pad=-MREF_C)
    r[1, 2] = _block(xy1_t[2] * dgt_t - MREF_C * (1.0 - mg_t), cfg,
                     pad=-MREF_C)
    for c in range(3):
        rgb_b = np.asarray(rgb[b, c], np.float32)
        q[0, 3 + c] = q[1, 3 + c] = _block(rgb_b, cfg)
        r[0, 3 + c] = q[0, 3 + c]
        r[1, 3 + c] = _block(np.asarray(rgb[tb, c], np.float32), cfg)
    return {"q_d": q, "r_d": r, "selw_d": make_selw()}


def make_in_maps(rgb, depth, depth_gt, depth_mask, depth_gt_mask, xy1_grid, Ts,
                 cfg, n_cores=8):
    return [host_precompute(rgb, depth, depth_gt, depth_mask, depth_gt_mask,
                            xy1_grid, Ts, cfg, b) for b in range(n_cores)]


_CACHED = {}


def _get_nc(cfg_key=(352, 1216, 32)):
    if cfg_key not in _CACHED:
        cfg = Cfg(*cfg_key)
        nc = bass.Bass()
        emit(nc, cfg)
        _CACHED[cfg_key] = (nc, cfg)
    return _CACHED[cfg_key]


def kernel(rgb, depth, depth_gt, depth_mask, depth_gt_mask, xy1_grid, Ts,
           **run_kwargs):
    from concourse.bass_utils import run_bass_kernel_spmd
    nc, cfg = _get_nc()
    maps = make_in_maps(rgb, depth, depth_gt, depth_mask, depth_gt_mask,
                        xy1_grid, Ts, cfg)
    res = run_bass_kernel_spmd(nc, maps, list(range(8)), **run_kwargs)
    garbage = cfg.garbage_const()
    total = np.float64(0.0)
    for r in res.results:
        total += np.float64(r["out_d"][:, 0].sum()) - garbage
    n_gt = max(np.asarray(depth_gt_mask, np.float64).sum(), 1.0)
    loss = -total / n_gt
    kernel.last_results = res
    return np.float32(loss)
